# revision 1
# baseline (speedup 1.0000x reference)
"""Trainium2 Bass kernel for DenseEquivariantMatrix.

Math:  out[b, fo, g] = sum_{fi,h} x[b, fi, h] * kernel[fo, fi, pt[h, g]] + bias[fo]

A B x K x N matmul (K = fi*h = 8192, N = fo*g = 8192) whose weight matrix is a
gather of 32x32 blocks from the kernel table.  Sharding: tensor-parallel over
the output n_symm dim (32 g's per core, 8 cores).

Per-core dataflow (all dtypes float32r = fp32 bits, FP22 multiply, fp32 accum):
  - indirect-DMA gather, one whole 4KB kernel-table block per partition:
    G[h_loc, (g, fi, fo)] = KT[pt[h, g]]; 32 gathers per h-half (one per g),
    offsets are raw pt values (coef = 1024 from the table AP shape).
  - matmul rhs is a strided 3D AP into G at fixed fi: [h x (g,16) x (fo,32)]
    = 512 columns; lhsT is an X^T chunk [h x b] (host-pretransposed layout).
  - K accumulated in PSUM over 32 fi-chunks per h-half; h-half 2 adds bias
    via a K=1 ones^T @ bias_row matmul and accumulates into DRAM with a
    SWDGE accum_op=add DMA.
"""

import os
import numpy as np

B = 2048
F_IN = 32
F_OUT = 32
H = 256  # n_symm (contraction copy)
G = 256  # n_symm (output copy)
N_CORES = 8
G_CORE = G // N_CORES  # 32
K = F_IN * H  # 8192
N_COLS = G_CORE * F_OUT  # 1024 per core, cols ordered (g_local, fo)
BLK = F_IN * F_OUT  # 1024 elements per kernel-table block

TRACE = bool(int(os.environ.get("KERNEL_TRACE", "0")))
LAST_RESULTS = None

_PROGRAM = None


def _build_program():
    import concourse.bacc as bacc
    import concourse.bass as bass
    import concourse.mybir as mybir
    import concourse.tile as tile

    f32 = mybir.dt.float32
    f32r = mybir.dt.float32r
    i32 = mybir.dt.int32

    nc = bacc.Bacc(
        "TRN2", target_bir_lowering=False, debug=False, num_devices=N_CORES
    )

    # host-tiled X^T: xt[hc, m, p, fi, j] = x[m*128+j, fi, hc*128+p]
    # -> per (hc, m) slab, each partition p reads 16KB contiguous
    xt = nc.dram_tensor(
        "xt", (2, B // 128, 128, F_IN, 128), f32r, kind="ExternalInput"
    ).ap()
    kt = nc.dram_tensor("kt", (H, BLK), f32r, kind="ExternalInput").ap()
    ptg = nc.dram_tensor("ptg", (H, G_CORE), i32, kind="ExternalInput").ap()
    biasrow = nc.dram_tensor("biasrow", (1, N_COLS), f32r, kind="ExternalInput").ap()
    onesrow = nc.dram_tensor("onesrow", (1, 128), f32r, kind="ExternalInput").ap()
    out = nc.dram_tensor("out", (B, N_COLS), f32, kind="ExternalOutput").ap()

    M_BLK = B // 128  # 16

    with tile.TileContext(nc) as tc:
        with (
            tc.tile_pool(name="const", bufs=1) as const_pool,
            tc.tile_pool(name="g", bufs=2) as g_pool,
            tc.tile_pool(name="x", bufs=3) as x_pool,
            tc.tile_pool(name="o", bufs=2) as o_pool,
            tc.tile_pool(name="psum", bufs=2, space="PSUM") as psum_pool,
        ):
            # pts[p, hc*32+g] = pt[hc*128+p, g]
            pts = const_pool.tile([128, 2 * G_CORE], i32, tag="pts")
            nc.sync.dma_start(
                pts[:].rearrange("p (hc g) -> p hc g", hc=2),
                ptg.rearrange("(hc p) g -> p hc g", p=128),
            )
            bias_t = const_pool.tile([1, N_COLS], f32r, tag="bias")
            nc.sync.dma_start(bias_t[:], biasrow[:])
            ones_t = const_pool.tile([1, 128], f32r, tag="ones")
            nc.sync.dma_start(ones_t[:], onesrow[:])

            NH = G_CORE // 2  # 16 g's per n-half panel
            for hc in range(2):
                G4s = []
                for nh in range(2):
                    Gt = g_pool.tile([128, NH * BLK], f32r, tag="G")
                    for g in range(NH):
                        gg = hc * G_CORE + nh * NH + g
                        nc.gpsimd.indirect_dma_start(
                            out=Gt[:, g * BLK : (g + 1) * BLK],
                            out_offset=None,
                            in_=kt[:],
                            in_offset=bass.IndirectOffsetOnAxis(
                                ap=pts[:, gg : gg + 1], axis=0
                            ),
                        )
                    G4s.append(
                        Gt[:].rearrange("p (g fi fo) -> p g fi fo", g=NH, fi=F_IN)
                    )

                for m in range(M_BLK):
                    xsl = x_pool.tile([128, F_IN * 128], f32r, tag="x")
                    nc.sync.dma_start(
                        xsl[:],
                        xt[hc, m].rearrange("p fi j -> p (fi j)"),
                    )
                    ps = psum_pool.tile([128, N_COLS], f32, tag="ps")
                    if m == 0:
                        # panel-then-fi order: start computing after the
                        # first 16-g panel lands instead of both
                        for nh in range(2):
                            for fi in range(F_IN):
                                last = hc == 0 and fi == F_IN - 1
                                nc.tensor.matmul(
                                    ps[:, nh * 512 : (nh + 1) * 512],
                                    lhsT=xsl[:, fi * 128 : (fi + 1) * 128],
                                    rhs=G4s[nh][:, :, fi, :],
                                    start=(fi == 0),
                                    stop=last,
                                )
                    else:
                        for fi in range(F_IN):
                            lhsT = xsl[:, fi * 128 : (fi + 1) * 128]
                            last = hc == 0 and fi == F_IN - 1
                            nc.tensor.matmul(
                                ps[:, 0:512],
                                lhsT=lhsT,
                                rhs=G4s[0][:, :, fi, :],
                                start=(fi == 0),
                                stop=last,
                            )
                            nc.tensor.matmul(
                                ps[:, 512:1024],
                                lhsT=lhsT,
                                rhs=G4s[1][:, :, fi, :],
                                start=(fi == 0),
                                stop=last,
                            )
                    if hc == 1:
                        nc.tensor.matmul(
                            ps[:, 0:512],
                            lhsT=ones_t[:],
                            rhs=bias_t[:, 0:512],
                            start=False,
                            stop=True,
                        )
                        nc.tensor.matmul(
                            ps[:, 512:1024],
                            lhsT=ones_t[:],
                            rhs=bias_t[:, 512:1024],
                            start=False,
                            stop=True,
                        )
                    ot = o_pool.tile([128, N_COLS], f32, tag="o")
                    nc.vector.tensor_copy(ot[:], ps[:])
                    if hc == 0:
                        nc.sync.dma_start(
                            out[m * 128 : (m + 1) * 128, :], ot[:]
                        )
                    else:
                        nc.gpsimd.dma_start(
                            out[m * 128 : (m + 1) * 128, :],
                            ot[:],
                            accum_op=mybir.AluOpType.add,
                        )

    nc.compile()
    return nc


def _get_program():
    global _PROGRAM
    if _PROGRAM is None:
        _PROGRAM = _build_program()
    return _PROGRAM


def kernel(x, kernel, bias, product_table):
    global LAST_RESULTS
    from concourse import bass_utils

    x = np.asarray(x, dtype=np.float32)
    kernel = np.asarray(kernel, dtype=np.float32)
    bias = np.asarray(bias, dtype=np.float32)
    product_table = np.asarray(product_table, dtype=np.int32)

    nc = _get_program()

    # host-tiled X^T: xt[hc, m, p, fi, j] = x[m*128+j, fi, hc*128+p]
    xt = np.ascontiguousarray(
        x.reshape(B // 128, 128, F_IN, 2, 128).transpose(3, 0, 4, 2, 1)
    )
    # kernel table KT[k][fi][fo]
    kt = np.ascontiguousarray(kernel.transpose(2, 1, 0)).reshape(H, BLK)
    bias_row = np.ascontiguousarray(np.tile(bias, G_CORE)[None, :])
    ones_row = np.ones((1, 128), np.float32)

    in_maps = []
    for c in range(N_CORES):
        in_maps.append(
            {
                "xt": xt,
                "kt": kt,
                "ptg": np.ascontiguousarray(
                    product_table[:, c * G_CORE : (c + 1) * G_CORE]
                ),
                "biasrow": bias_row,
                "onesrow": ones_row,
            }
        )

    res = bass_utils.run_bass_kernel_spmd(
        nc,
        in_maps,
        core_ids=list(range(N_CORES)),
        trace=TRACE,
        trace_cores=[0] if TRACE else None,
        tmpdir=os.environ.get("KERNEL_TMPDIR") or None,
    )
    LAST_RESULTS = res

    # per-core cols are (g_local, fo); assemble to (B, F_OUT, G)
    parts = [
        res.results[c]["out"].reshape(B, G_CORE, F_OUT).transpose(0, 2, 1)
        for c in range(N_CORES)
    ]
    return np.ascontiguousarray(np.concatenate(parts, axis=2), dtype=np.float32)



# revision 2
# speedup vs baseline: 1.4228x; 1.4228x over previous
"""Trainium2 Bass kernel for DenseEquivariantMatrix.

Math:  out[b, fo, g] = sum_{fi,h} x[b, fi, h] * kernel[fo, fi, pt[h, g]] + bias[fo]

A B x K x N matmul (K = fi*h = 8192, N = fo*g = 8192) whose weight matrix is a
gather of 32x32 blocks from the kernel table.  Sharding: tensor-parallel over
the output n_symm dim (32 g's per core, 8 cores).

Per-core dataflow (bf16 operands, fp32 PSUM accumulation):
  - host converts x (pre-transposed) and the kernel table to bf16; the
    product-table gather runs on-device as 64 indirect DMAs (one whole 2KB
    bf16 kernel-table block per partition), hc-major so the first half-G
    lands early.
  - matmul rhs is a strided 3D AP into G at fixed (hc, fi): [h x (g,16) x
    (fo,32)] = 512 columns; lhsT is an X^T chunk [h x b].  bf16 weights get
    FWL (fast weight load), so the per-matmul LDWEIGHTS mostly hides under
    the 512-col stream (~213ns at 2.4GHz).
  - K accumulated in PSUM over 32 fi-chunks per h-half; h-half 2 adds bias
    via a K=1 ones^T @ bias_row matmul and accumulates into DRAM with a
    SWDGE accum_op=add DMA.
"""

import os
import numpy as np

B = 2048
F_IN = 32
F_OUT = 32
H = 256  # n_symm (contraction copy)
G = 256  # n_symm (output copy)
N_CORES = 8
G_CORE = G // N_CORES  # 32
K = F_IN * H  # 8192
N_COLS = G_CORE * F_OUT  # 1024 per core, cols ordered (g_local, fo)
BLK = F_IN * F_OUT  # 1024 elements per kernel-table block

TRACE = bool(int(os.environ.get("KERNEL_TRACE", "0")))
LAST_RESULTS = None

_PROGRAM = None


def _build_program():
    import concourse.bacc as bacc
    import concourse.bass as bass
    import concourse.mybir as mybir
    import concourse.tile as tile

    f32 = mybir.dt.float32
    bf16 = mybir.dt.bfloat16
    i32 = mybir.dt.int32

    nc = bacc.Bacc(
        "TRN2", target_bir_lowering=False, debug=False, num_devices=N_CORES
    )

    # host-tiled X^T: xt[hc, m, p, fi, j] = x[m*128+j, fi, hc*128+p]
    # -> per (hc, m) slab, each partition p reads 8KB contiguous (bf16)
    xt = nc.dram_tensor(
        "xt", (2, B // 128, 128, F_IN, 128), bf16, kind="ExternalInput"
    ).ap()
    kt = nc.dram_tensor("kt", (H, BLK), bf16, kind="ExternalInput").ap()
    ptg = nc.dram_tensor("ptg", (H, G_CORE), i32, kind="ExternalInput").ap()
    biasrow = nc.dram_tensor("biasrow", (1, N_COLS), bf16, kind="ExternalInput").ap()
    onesrow = nc.dram_tensor("onesrow", (1, 128), bf16, kind="ExternalInput").ap()
    out = nc.dram_tensor("out", (B, N_COLS), f32, kind="ExternalOutput").ap()

    M_BLK = B // 128  # 16

    with tile.TileContext(nc) as tc:
        with (
            tc.tile_pool(name="const", bufs=1) as const_pool,
            tc.tile_pool(name="g", bufs=1) as g_pool,
            tc.tile_pool(name="x", bufs=3) as x_pool,
            tc.tile_pool(name="o", bufs=2) as o_pool,
            tc.tile_pool(name="psum", bufs=2, space="PSUM") as psum_pool,
        ):
            # pts[p, hc*32+g] = pt[hc*128+p, g]
            pts = const_pool.tile([128, 2 * G_CORE], i32, tag="pts")
            nc.sync.dma_start(
                pts[:].rearrange("p (hc g) -> p hc g", hc=2),
                ptg.rearrange("(hc p) g -> p hc g", p=128),
            )
            bias_t = const_pool.tile([1, N_COLS], bf16, tag="bias")
            nc.sync.dma_start(bias_t[:], biasrow[:])
            ones_t = const_pool.tile([1, 128], bf16, tag="ones")
            nc.sync.dma_start(ones_t[:], onesrow[:])

            # whole G resident in SBUF: 128KB/partition in bf16
            Gt = g_pool.tile([128, 2 * G_CORE * BLK], bf16, tag="G")
            for hc in range(2):
                for g in range(G_CORE):
                    gg = hc * G_CORE + g
                    nc.gpsimd.indirect_dma_start(
                        out=Gt[:, gg * BLK : (gg + 1) * BLK],
                        out_offset=None,
                        in_=kt[:],
                        in_offset=bass.IndirectOffsetOnAxis(
                            ap=pts[:, gg : gg + 1], axis=0
                        ),
                    )
            G4 = Gt[:].rearrange(
                "p (hc g fi fo) -> p hc g fi fo", hc=2, g=G_CORE, fi=F_IN
            )

            NH = G_CORE // 2  # 16 g's per n-half panel

            for hc in range(2):
                for m in range(M_BLK):
                    xsl = x_pool.tile([128, F_IN * 128], bf16, tag="x")
                    nc.sync.dma_start(
                        xsl[:],
                        xt[hc, m].rearrange("p fi j -> p (fi j)"),
                    )
                    ps = psum_pool.tile([128, N_COLS], f32, tag="ps")
                    if hc == 0 and m == 0:
                        # panel-then-fi order: start computing after the
                        # first 16-g panel's gathers land instead of all 32
                        for nh in range(2):
                            for fi in range(F_IN):
                                nc.tensor.matmul(
                                    ps[:, nh * 512 : (nh + 1) * 512],
                                    lhsT=xsl[:, fi * 128 : (fi + 1) * 128],
                                    rhs=G4[:, hc, nh * NH : (nh + 1) * NH, fi, :],
                                    start=(fi == 0),
                                    stop=(fi == F_IN - 1),
                                )
                    else:
                        for fi in range(F_IN):
                            lhsT = xsl[:, fi * 128 : (fi + 1) * 128]
                            last = hc == 0 and fi == F_IN - 1
                            nc.tensor.matmul(
                                ps[:, 0:512],
                                lhsT=lhsT,
                                rhs=G4[:, hc, 0:NH, fi, :],
                                start=(fi == 0),
                                stop=last,
                            )
                            nc.tensor.matmul(
                                ps[:, 512:1024],
                                lhsT=lhsT,
                                rhs=G4[:, hc, NH : 2 * NH, fi, :],
                                start=(fi == 0),
                                stop=last,
                            )
                    if hc == 1:
                        nc.tensor.matmul(
                            ps[:, 0:512],
                            lhsT=ones_t[:],
                            rhs=bias_t[:, 0:512],
                            start=False,
                            stop=True,
                        )
                        nc.tensor.matmul(
                            ps[:, 512:1024],
                            lhsT=ones_t[:],
                            rhs=bias_t[:, 512:1024],
                            start=False,
                            stop=True,
                        )
                    ot = o_pool.tile([128, N_COLS], f32, tag="o")
                    nc.vector.tensor_copy(ot[:], ps[:])
                    if hc == 0:
                        nc.sync.dma_start(
                            out[m * 128 : (m + 1) * 128, :], ot[:]
                        )
                    else:
                        nc.gpsimd.dma_start(
                            out[m * 128 : (m + 1) * 128, :],
                            ot[:],
                            accum_op=mybir.AluOpType.add,
                        )

    nc.compile()
    return nc


def _get_program():
    global _PROGRAM
    if _PROGRAM is None:
        _PROGRAM = _build_program()
    return _PROGRAM


def kernel(x, kernel, bias, product_table):
    global LAST_RESULTS
    import ml_dtypes
    from concourse import bass_utils

    bf = ml_dtypes.bfloat16
    x = np.asarray(x, dtype=np.float32)
    kernel = np.asarray(kernel, dtype=np.float32)
    bias = np.asarray(bias, dtype=np.float32)
    product_table = np.asarray(product_table, dtype=np.int32)

    nc = _get_program()

    # host-tiled X^T: xt[hc, m, p, fi, j] = x[m*128+j, fi, hc*128+p]
    xt = np.ascontiguousarray(
        x.astype(bf).reshape(B // 128, 128, F_IN, 2, 128).transpose(3, 0, 4, 2, 1)
    )
    # kernel table KT[k][fi][fo]
    kt = np.ascontiguousarray(
        kernel.astype(bf).transpose(2, 1, 0)
    ).reshape(H, BLK)
    bias_row = np.ascontiguousarray(np.tile(bias, G_CORE)[None, :].astype(bf))
    ones_row = np.ones((1, 128), np.float32).astype(bf)

    in_maps = []
    for c in range(N_CORES):
        in_maps.append(
            {
                "xt": xt,
                "kt": kt,
                "ptg": np.ascontiguousarray(
                    product_table[:, c * G_CORE : (c + 1) * G_CORE]
                ),
                "biasrow": bias_row,
                "onesrow": ones_row,
            }
        )

    res = bass_utils.run_bass_kernel_spmd(
        nc,
        in_maps,
        core_ids=list(range(N_CORES)),
        trace=TRACE,
        trace_cores=[0] if TRACE else None,
        tmpdir=os.environ.get("KERNEL_TMPDIR") or None,
    )
    LAST_RESULTS = res

    # per-core cols are (g_local, fo); assemble to (B, F_OUT, G)
    parts = [
        res.results[c]["out"].reshape(B, G_CORE, F_OUT).transpose(0, 2, 1)
        for c in range(N_CORES)
    ]
    return np.ascontiguousarray(np.concatenate(parts, axis=2), dtype=np.float32)


# revision 6
# speedup vs baseline: 1.4771x; 1.0382x over previous
"""Trainium2 Bass kernel for DenseEquivariantMatrix.

Math:  out[b, fo, g] = sum_{fi,h} x[b, fi, h] * kernel[fo, fi, pt[h, g]] + bias[fo]

A B x K x N matmul (K = fi*h = 8192, N = fo*g = 8192) whose weight matrix is a
gather of 32x32 blocks from the kernel table.  Sharding: tensor-parallel over
the output n_symm dim (32 g's per core, 8 cores).

Per-core dataflow (bf16 operands, fp32 PSUM accumulation):
  - host converts x (pre-transposed) and the kernel table to bf16; the
    product-table gather runs on-device as 64 indirect DMAs (one whole 2KB
    bf16 kernel-table block per partition), hc-major.  Gather issue is
    ~1.4us each on gpsimd, so the matmul schedule chases the gather front:
    slab 0 starts on an 8-g quarter panel, pass 1 (h-half 1) runs as two
    half-width panel passes over all 16 b-slabs, and pass 2 (h-half 2)
    runs full-width once all of G is resident.
  - matmul rhs is a strided 3D AP into G at fixed (hc, fi): [h x g x fo]
    columns; lhsT is an X^T chunk [h x b].  bf16 weights get FWL, so the
    per-matmul LDWEIGHTS (~97ns) hides under the 512-col stream (~216ns).
  - K accumulated in PSUM over 32 fi-chunks per h-half; h-half 2 adds bias
    via a K=1 ones^T @ bias_row matmul and accumulates into DRAM with a
    SWDGE accum_op=add DMA.
"""

import os
import numpy as np

B = 2048
F_IN = 32
F_OUT = 32
H = 256  # n_symm (contraction copy)
G = 256  # n_symm (output copy)
N_CORES = 8
G_CORE = G // N_CORES  # 32
K = F_IN * H  # 8192
N_COLS = G_CORE * F_OUT  # 1024 per core, cols ordered (g_local, fo)
BLK = F_IN * F_OUT  # 1024 elements per kernel-table block

TRACE = bool(int(os.environ.get("KERNEL_TRACE", "0")))
LAST_RESULTS = None

_PROGRAM = None


def _build_program():
    import concourse.bacc as bacc
    import concourse.bass as bass
    import concourse.mybir as mybir
    import concourse.tile as tile

    f32 = mybir.dt.float32
    bf16 = mybir.dt.bfloat16
    i32 = mybir.dt.int32

    nc = bacc.Bacc(
        "TRN2", target_bir_lowering=False, debug=False, num_devices=N_CORES
    )

    # host-tiled X^T: xt[hc, m, p, fi, j] = x[m*128+j, fi, hc*128+p]
    # -> per (hc, m) slab, each partition p reads 8KB contiguous (bf16)
    xt = nc.dram_tensor(
        "xt", (2, B // 128, 128, F_IN, 128), bf16, kind="ExternalInput"
    ).ap()
    kt = nc.dram_tensor("kt", (H, BLK), bf16, kind="ExternalInput").ap()
    ptg = nc.dram_tensor("ptg", (H, G_CORE), i32, kind="ExternalInput").ap()
    biasrow = nc.dram_tensor("biasrow", (1, N_COLS), bf16, kind="ExternalInput").ap()
    onesrow = nc.dram_tensor("onesrow", (1, 128), bf16, kind="ExternalInput").ap()
    out = nc.dram_tensor("out", (B, N_COLS), f32, kind="ExternalOutput").ap()

    M_BLK = B // 128  # 16

    with tile.TileContext(nc) as tc:
        with (
            tc.tile_pool(name="const", bufs=1) as const_pool,
            tc.tile_pool(name="g", bufs=1) as g_pool,
            tc.tile_pool(name="x", bufs=4) as x_pool,
            tc.tile_pool(name="oh", bufs=3) as oh_pool,
            tc.tile_pool(name="of", bufs=2) as of_pool,
            tc.tile_pool(name="psh", bufs=3, space="PSUM") as psh_pool,
            tc.tile_pool(name="psf", bufs=2, space="PSUM") as psf_pool,
        ):
            # pts[p, hc*32+g] = pt[hc*128+p, g]
            pts = const_pool.tile([128, 2 * G_CORE], i32, tag="pts")
            nc.sync.dma_start(
                pts[:].rearrange("p (hc g) -> p hc g", hc=2),
                ptg.rearrange("(hc p) g -> p hc g", p=128),
            )
            bias_t = const_pool.tile([1, N_COLS], bf16, tag="bias")
            nc.sync.dma_start(bias_t[:], biasrow[:])
            ones_t = const_pool.tile([1, 128], bf16, tag="ones")
            nc.sync.dma_start(ones_t[:], onesrow[:])

            # whole G resident in SBUF: 128KB/partition in bf16
            Gt = g_pool.tile([128, 2 * G_CORE * BLK], bf16, tag="G")
            for hc in range(2):
                for g in range(G_CORE):
                    gg = hc * G_CORE + g
                    nc.gpsimd.indirect_dma_start(
                        out=Gt[:, gg * BLK : (gg + 1) * BLK],
                        out_offset=None,
                        in_=kt[:],
                        in_offset=bass.IndirectOffsetOnAxis(
                            ap=pts[:, gg : gg + 1], axis=0
                        ),
                    )
            G4 = Gt[:].rearrange(
                "p (hc g fi fo) -> p hc g fi fo", hc=2, g=G_CORE, fi=F_IN
            )

            def load_x(hc, m):
                xsl = x_pool.tile([128, F_IN * 128], bf16, tag="x")
                nc.sync.dma_start(
                    xsl[:], xt[hc, m].rearrange("p fi j -> p (fi j)")
                )
                return xsl

            def mm_panel(ps_ap, xsl, hc, g0, g1, start, stop):
                # accumulate x^T @ G[hc, g0:g1] over all fi into ps_ap
                for fi in range(F_IN):
                    nc.tensor.matmul(
                        ps_ap,
                        lhsT=xsl[:, fi * 128 : (fi + 1) * 128],
                        rhs=G4[:, hc, g0:g1, fi, :],
                        start=start and fi == 0,
                        stop=stop and fi == F_IN - 1,
                    )

            def evac_half(m, c0, cols, ps_ap):
                ot = oh_pool.tile([128, 512], f32, tag="oh")
                nc.vector.tensor_copy(ot[:, 0:cols], ps_ap)
                nc.sync.dma_start(
                    out[m * 128 : (m + 1) * 128, c0 : c0 + cols], ot[:, 0:cols]
                )

            # ---- pass 1 (hc=0): chase the gather front ----
            # slab 0 on 8-g quarter panels (ready after 8 gathers)
            for q in range(2):
                xsl = load_x(0, 0)
                ps = psh_pool.tile([128, 512], f32, tag="psh")
                mm_panel(ps[:, 0:256], xsl, 0, q * 8, (q + 1) * 8, True, True)
                evac_half(0, q * 256, 256, ps[:, 0:256])
            # panel A (g 0:16) for slabs 1..15, then panel B (g 16:32) all
            for half, m_lo in ((0, 1), (1, 0)):
                for m in range(m_lo, M_BLK):
                    xsl = load_x(0, m)
                    ps = psh_pool.tile([128, 512], f32, tag="psh")
                    mm_panel(ps[:], xsl, 0, half * 16, (half + 1) * 16, True, True)
                    evac_half(m, half * 512, 512, ps[:])
            # slab 0 panel B second quarter (cols 512:1024 handled above for
            # half=1; quarters covered cols 0:512 only)

            # ---- pass 2 (hc=1): full width + bias, accumulate into DRAM ----
            for m in range(M_BLK):
                xsl = load_x(1, m)
                if m < M_BLK - 1:
                    ps = psf_pool.tile([128, N_COLS], f32, tag="psf")
                    for fi in range(F_IN):
                        lhsT = xsl[:, fi * 128 : (fi + 1) * 128]
                        nc.tensor.matmul(
                            ps[:, 0:512], lhsT=lhsT,
                            rhs=G4[:, 1, 0:16, fi, :],
                            start=(fi == 0), stop=False,
                        )
                        nc.tensor.matmul(
                            ps[:, 512:1024], lhsT=lhsT,
                            rhs=G4[:, 1, 16:32, fi, :],
                            start=(fi == 0), stop=False,
                        )
                    nc.tensor.matmul(
                        ps[:, 0:512], lhsT=ones_t[:], rhs=bias_t[:, 0:512],
                        start=False, stop=True,
                    )
                    nc.tensor.matmul(
                        ps[:, 512:1024], lhsT=ones_t[:], rhs=bias_t[:, 512:1024],
                        start=False, stop=True,
                    )
                    ot = of_pool.tile([128, N_COLS], f32, tag="of")
                    nc.vector.tensor_copy(ot[:], ps[:])
                    nc.gpsimd.dma_start(
                        out[m * 128 : (m + 1) * 128, :], ot[:],
                        accum_op=mybir.AluOpType.add,
                    )
                else:
                    # last slab: column-split so the first half's
                    # evacuation overlaps the second half's matmuls
                    for half in range(2):
                        ps = psh_pool.tile([128, 512], f32, tag="psh")
                        mm_panel(
                            ps[:], xsl, 1, half * 16, (half + 1) * 16, True, False
                        )
                        nc.tensor.matmul(
                            ps[:], lhsT=ones_t[:],
                            rhs=bias_t[:, half * 512 : (half + 1) * 512],
                            start=False, stop=True,
                        )
                        ot = oh_pool.tile([128, 512], f32, tag="oh")
                        nc.vector.tensor_copy(ot[:], ps[:])
                        nc.gpsimd.dma_start(
                            out[m * 128 : (m + 1) * 128,
                                half * 512 : (half + 1) * 512],
                            ot[:], accum_op=mybir.AluOpType.add,
                        )

    nc.compile()
    return nc


def _get_program():
    global _PROGRAM
    if _PROGRAM is None:
        _PROGRAM = _build_program()
    return _PROGRAM


def kernel(x, kernel, bias, product_table):
    global LAST_RESULTS
    import ml_dtypes
    from concourse import bass_utils

    bf = ml_dtypes.bfloat16
    x = np.asarray(x, dtype=np.float32)
    kernel = np.asarray(kernel, dtype=np.float32)
    bias = np.asarray(bias, dtype=np.float32)
    product_table = np.asarray(product_table, dtype=np.int32)

    nc = _get_program()

    # host-tiled X^T: xt[hc, m, p, fi, j] = x[m*128+j, fi, hc*128+p]
    xt = np.ascontiguousarray(
        x.astype(bf).reshape(B // 128, 128, F_IN, 2, 128).transpose(3, 0, 4, 2, 1)
    )
    # kernel table KT[k][fi][fo]
    kt = np.ascontiguousarray(
        kernel.astype(bf).transpose(2, 1, 0)
    ).reshape(H, BLK)
    bias_row = np.ascontiguousarray(np.tile(bias, G_CORE)[None, :].astype(bf))
    ones_row = np.ones((1, 128), np.float32).astype(bf)

    in_maps = []
    for c in range(N_CORES):
        in_maps.append(
            {
                "xt": xt,
                "kt": kt,
                "ptg": np.ascontiguousarray(
                    product_table[:, c * G_CORE : (c + 1) * G_CORE]
                ),
                "biasrow": bias_row,
                "onesrow": ones_row,
            }
        )

    res = bass_utils.run_bass_kernel_spmd(
        nc,
        in_maps,
        core_ids=list(range(N_CORES)),
        trace=TRACE,
        trace_cores=[0] if TRACE else None,
        tmpdir=os.environ.get("KERNEL_TMPDIR") or None,
    )
    LAST_RESULTS = res

    # per-core cols are (g_local, fo); assemble to (B, F_OUT, G)
    parts = [
        res.results[c]["out"].reshape(B, G_CORE, F_OUT).transpose(0, 2, 1)
        for c in range(N_CORES)
    ]
    return np.ascontiguousarray(np.concatenate(parts, axis=2), dtype=np.float32)


# revision 7
# speedup vs baseline: 1.5145x; 1.0253x over previous
"""Trainium2 Bass kernel for DenseEquivariantMatrix.

Math:  out[b, fo, g] = sum_{fi,h} x[b, fi, h] * kernel[fo, fi, pt[h, g]] + bias[fo]

A B x K x N matmul (K = fi*h = 8192, N = fo*g = 8192) whose weight matrix is a
gather of 32x32 blocks from the kernel table.  Sharding: tensor-parallel over
the output n_symm dim (32 g's per core, 8 cores).

Per-core dataflow (bf16 operands, fp32 PSUM accumulation):
  - host converts x (pre-transposed) and the kernel table to bf16; the
    product-table gather runs on-device as 64 indirect DMAs (one whole 2KB
    bf16 kernel-table block per partition), hc-major.  Gather issue is
    ~1.4us each on gpsimd, so the matmul schedule chases the gather front:
    slab 0 starts on an 8-g quarter panel, pass 1 (h-half 1) runs as two
    half-width panel passes over all 16 b-slabs, and pass 2 (h-half 2)
    runs full-width once all of G is resident.
  - matmul rhs is a strided 3D AP into G at fixed (hc, fi): [h x g x fo]
    columns; lhsT is an X^T chunk [h x b].  bf16 weights get FWL, so the
    per-matmul LDWEIGHTS (~97ns) hides under the 512-col stream (~216ns).
  - K accumulated in PSUM over 32 fi-chunks per h-half; h-half 2 adds bias
    via a K=1 ones^T @ bias_row matmul and accumulates into DRAM with a
    SWDGE accum_op=add DMA.
"""

import os
import numpy as np

B = 2048
F_IN = 32
F_OUT = 32
H = 256  # n_symm (contraction copy)
G = 256  # n_symm (output copy)
N_CORES = 8
G_CORE = G // N_CORES  # 32
K = F_IN * H  # 8192
N_COLS = G_CORE * F_OUT  # 1024 per core, cols ordered (g_local, fo)
BLK = F_IN * F_OUT  # 1024 elements per kernel-table block

TRACE = bool(int(os.environ.get("KERNEL_TRACE", "0")))
LAST_RESULTS = None

_PROGRAM = None


def _build_program():
    import concourse.bacc as bacc
    import concourse.bass as bass
    import concourse.mybir as mybir
    import concourse.tile as tile

    f32 = mybir.dt.float32
    bf16 = mybir.dt.bfloat16
    i32 = mybir.dt.int32

    nc = bacc.Bacc(
        "TRN2", target_bir_lowering=False, debug=False, num_devices=N_CORES
    )

    # host-tiled X^T: xt[hc, m, p, fi, j] = x[m*128+j, fi, hc*128+p]
    # -> per (hc, m) slab, each partition p reads 8KB contiguous (bf16)
    xt = nc.dram_tensor(
        "xt", (2, B // 128, 128, F_IN, 128), bf16, kind="ExternalInput"
    ).ap()
    kt = nc.dram_tensor("kt", (H, BLK), bf16, kind="ExternalInput").ap()
    ptg = nc.dram_tensor("ptg", (H, G_CORE), i32, kind="ExternalInput").ap()
    biasrow = nc.dram_tensor("biasrow", (1, N_COLS), f32, kind="ExternalInput").ap()
    out = nc.dram_tensor("out", (B, N_COLS), f32, kind="ExternalOutput").ap()

    M_BLK = B // 128  # 16

    with tile.TileContext(nc) as tc:
        with (
            tc.tile_pool(name="const", bufs=1) as const_pool,
            tc.tile_pool(name="g", bufs=1) as g_pool,
            tc.tile_pool(name="x", bufs=4) as x_pool,
            tc.tile_pool(name="oh", bufs=3) as oh_pool,
            tc.tile_pool(name="of", bufs=2) as of_pool,
            tc.tile_pool(name="psh", bufs=3, space="PSUM") as psh_pool,
            tc.tile_pool(name="psf", bufs=2, space="PSUM") as psf_pool,
        ):
            # pts[p, hc*32+g] = pt[hc*128+p, g]
            pts = const_pool.tile([128, 2 * G_CORE], i32, tag="pts")
            nc.sync.dma_start(
                pts[:].rearrange("p (hc g) -> p hc g", hc=2),
                ptg.rearrange("(hc p) g -> p hc g", p=128),
            )
            # bias broadcast to all partitions, added during PSUM
            # evacuation on the vector engine (off the tensor engine)
            bias_sb = const_pool.tile([128, N_COLS], f32, tag="bias")
            nc.scalar.dma_start(bias_sb[:], biasrow.to_broadcast((128, N_COLS)))

            # whole G resident in SBUF: 128KB/partition in bf16
            Gt = g_pool.tile([128, 2 * G_CORE * BLK], bf16, tag="G")
            for hc in range(2):
                for g in range(G_CORE):
                    gg = hc * G_CORE + g
                    nc.gpsimd.indirect_dma_start(
                        out=Gt[:, gg * BLK : (gg + 1) * BLK],
                        out_offset=None,
                        in_=kt[:],
                        in_offset=bass.IndirectOffsetOnAxis(
                            ap=pts[:, gg : gg + 1], axis=0
                        ),
                    )
            G4 = Gt[:].rearrange(
                "p (hc g fi fo) -> p hc g fi fo", hc=2, g=G_CORE, fi=F_IN
            )

            def load_x(hc, m):
                xsl = x_pool.tile([128, F_IN * 128], bf16, tag="x")
                nc.sync.dma_start(
                    xsl[:], xt[hc, m].rearrange("p fi j -> p (fi j)")
                )
                return xsl

            def mm_panel(ps_ap, xsl, hc, g0, g1, start, stop):
                # accumulate x^T @ G[hc, g0:g1] over all fi into ps_ap
                for fi in range(F_IN):
                    nc.tensor.matmul(
                        ps_ap,
                        lhsT=xsl[:, fi * 128 : (fi + 1) * 128],
                        rhs=G4[:, hc, g0:g1, fi, :],
                        start=start and fi == 0,
                        stop=stop and fi == F_IN - 1,
                    )

            def evac_half(m, c0, cols, ps_ap):
                ot = oh_pool.tile([128, 512], f32, tag="oh")
                nc.vector.tensor_copy(ot[:, 0:cols], ps_ap)
                nc.sync.dma_start(
                    out[m * 128 : (m + 1) * 128, c0 : c0 + cols], ot[:, 0:cols]
                )

            # ---- pass 1 (hc=0): chase the gather front ----
            # slab 0 on 8-g quarter panels (ready after 8 gathers)
            for q in range(2):
                xsl = load_x(0, 0)
                ps = psh_pool.tile([128, 512], f32, tag="psh")
                mm_panel(ps[:, 0:256], xsl, 0, q * 8, (q + 1) * 8, True, True)
                evac_half(0, q * 256, 256, ps[:, 0:256])
            # panel A (g 0:16) for slabs 1..15, then panel B (g 16:32) all
            for half, m_lo in ((0, 1), (1, 0)):
                for m in range(m_lo, M_BLK - 1):
                    xsl = load_x(0, m)
                    ps = psh_pool.tile([128, 512], f32, tag="psh")
                    mm_panel(ps[:], xsl, 0, half * 16, (half + 1) * 16, True, True)
                    evac_half(m, half * 512, 512, ps[:])
            # slab 0 panel B second quarter (cols 512:1024 handled above for
            # half=1; quarters covered cols 0:512 only)

            # ---- pass 2 (hc=1): full width + bias, accumulate into DRAM ----
            for m in range(M_BLK - 1):
                xsl = load_x(1, m)
                ps = psf_pool.tile([128, N_COLS], f32, tag="psf")
                for fi in range(F_IN):
                    lhsT = xsl[:, fi * 128 : (fi + 1) * 128]
                    nc.tensor.matmul(
                        ps[:, 0:512], lhsT=lhsT,
                        rhs=G4[:, 1, 0:16, fi, :],
                        start=(fi == 0), stop=(fi == F_IN - 1),
                    )
                    nc.tensor.matmul(
                        ps[:, 512:1024], lhsT=lhsT,
                        rhs=G4[:, 1, 16:32, fi, :],
                        start=(fi == 0), stop=(fi == F_IN - 1),
                    )
                ot = of_pool.tile([128, N_COLS], f32, tag="of")
                nc.vector.tensor_add(ot[:], ps[:], bias_sb[:])
                nc.gpsimd.dma_start(
                    out[m * 128 : (m + 1) * 128, :], ot[:],
                    accum_op=mybir.AluOpType.add,
                )

            # ---- last slab full-K (both h-halves) with a plain final
            # write, column-split so colA's evacuation overlaps colB's
            # matmuls and the gpsimd accum queue drains early ----
            m = M_BLK - 1
            xsl0 = load_x(0, m)
            xsl1 = load_x(1, m)
            for half in range(2):
                ps = psh_pool.tile([128, 512], f32, tag="psh")
                for hc, xs in ((0, xsl0), (1, xsl1)):
                    for fi in range(F_IN):
                        nc.tensor.matmul(
                            ps[:],
                            lhsT=xs[:, fi * 128 : (fi + 1) * 128],
                            rhs=G4[:, hc, half * 16 : (half + 1) * 16, fi, :],
                            start=(hc == 0 and fi == 0),
                            stop=(hc == 1 and fi == F_IN - 1),
                        )
                ot = oh_pool.tile([128, 512], f32, tag="oh")
                nc.vector.tensor_add(
                    ot[:, 0:512], ps[:],
                    bias_sb[:, half * 512 : (half + 1) * 512],
                )
                nc.sync.dma_start(
                    out[m * 128 : (m + 1) * 128, half * 512 : (half + 1) * 512],
                    ot[:, 0:512],
                )

    nc.compile()
    return nc


def _get_program():
    global _PROGRAM
    if _PROGRAM is None:
        _PROGRAM = _build_program()
    return _PROGRAM


def kernel(x, kernel, bias, product_table):
    global LAST_RESULTS
    import ml_dtypes
    from concourse import bass_utils

    bf = ml_dtypes.bfloat16
    x = np.asarray(x, dtype=np.float32)
    kernel = np.asarray(kernel, dtype=np.float32)
    bias = np.asarray(bias, dtype=np.float32)
    product_table = np.asarray(product_table, dtype=np.int32)

    nc = _get_program()

    # host-tiled X^T: xt[hc, m, p, fi, j] = x[m*128+j, fi, hc*128+p]
    xt = np.ascontiguousarray(
        x.astype(bf).reshape(B // 128, 128, F_IN, 2, 128).transpose(3, 0, 4, 2, 1)
    )
    # kernel table KT[k][fi][fo]
    kt = np.ascontiguousarray(
        kernel.astype(bf).transpose(2, 1, 0)
    ).reshape(H, BLK)
    bias_row = np.ascontiguousarray(np.tile(bias, G_CORE)[None, :])

    in_maps = []
    for c in range(N_CORES):
        in_maps.append(
            {
                "xt": xt,
                "kt": kt,
                "ptg": np.ascontiguousarray(
                    product_table[:, c * G_CORE : (c + 1) * G_CORE]
                ),
                "biasrow": bias_row,
            }
        )

    res = bass_utils.run_bass_kernel_spmd(
        nc,
        in_maps,
        core_ids=list(range(N_CORES)),
        trace=TRACE,
        trace_cores=[0] if TRACE else None,
        tmpdir=os.environ.get("KERNEL_TMPDIR") or None,
    )
    LAST_RESULTS = res

    # per-core cols are (g_local, fo); assemble to (B, F_OUT, G)
    parts = [
        res.results[c]["out"].reshape(B, G_CORE, F_OUT).transpose(0, 2, 1)
        for c in range(N_CORES)
    ]
    return np.ascontiguousarray(np.concatenate(parts, axis=2), dtype=np.float32)
